# revision 32
# baseline (speedup 1.0000x reference)
"""Distributed multi-head attention kernel for trn2 (8 NeuronCores), v3.

Problem: B=2, N=4096, C=768, H=8 heads, Dh=96.
    qkv = x @ w_qkv ; per-head softmax(q k^T / sqrt(Dh)) v ; out @ w_proj + b_proj

Sharding (data parallel on B, tensor parallel on heads):
    core i -> batch b = i//4, heads (2*(i%4), 2*(i%4)+1)

v3 restructure vs v2:
  - PV matmul flipped to oT[d, q] orientation (lhsT = v-chunk stationary,
    rhs = p streaming 512 q-cols): one LDWEIGHTS per (qr, kc) instead of
    four -- on HW each ldweights costs ~108ns, so the old 97-col streams
    were weight-load-bound (2048 ldweights ~ 220us dispatched). The oT
    output is already the projection's lhsT layout, so the XBAR DMA
    transpose of v2 is gone as well.
  - the softmax denominator rides as PSUM row 96 of oT (ones-column in
    v); it is transposed to per-q-partition form with 4 tiny K=1
    matmuls per (head, qr), and normalization is applied per-head after
    the projection (DVE scale by 1/denom). The two heads' projections
    are combined by a DMA accumulate-write into the y bounce buffer.
  - per-qr (512-row) ReduceScatter on a per-qr bounce tile. v2's single
    y_bounce tile serialized each RS quarter against the next quarter's
    proj writes (tile-granularity WAR), stalling PE+ACT ~27us per RS.
  - weights are pre-transposed on host to [128, co, m] so every DMA
    descriptor is a contiguous per-partition strip (HWDGE, not SWDGE).
  - v-units compute both heads per ldweights (192-col streams).

Math notes (unchanged): scores ~ N(0,1) after the Dh^-0.5 scale (folded
into w_q on host), so softmax max-subtraction is skipped. Compute dtype
bf16 on the PE (f32 PSUM accumulation); ReduceScatter payload bf16.
"""

import numpy as np
import ml_dtypes

import concourse.bass as bass
import concourse.tile as tile
from concourse import mybir
from concourse.bass_utils import run_bass_kernel_spmd

# ---------------- problem constants (hardcoded per spec) ----------------
B, N, C, H, DH = 2, 4096, 768, 8, 96
HEADS_PER_CORE = 2
HD = HEADS_PER_CORE * DH  # 192
N_CORES = 8
GROUPS = [[0, 1, 2, 3], [4, 5, 6, 7]]
QR = 512  # query rows per oT accumulation (one PSUM bank of [97, 512])
N_QR = N // QR  # 8
KC = 128  # key chunk (contraction tile for PV)
N_KC = N // KC  # 32
SH = QR // 4  # rows per rank per RS chunk (128)

F32 = mybir.dt.float32
BF16 = mybir.dt.bfloat16
BF16_NP = ml_dtypes.bfloat16

_HOIST = True  # hoist inline waits (required for the walrus build; off for CoreSim)
_DEBUG_NO_RS = False  # replace ReduceScatter with a local copy (debug only)
_INTERLEAVE_QKV1 = True  # interleave head-1 QKV into head-0 attention

# exp offload: the ACT engine (1 elem/cycle/lane) would otherwise pace the
# whole score stream; route some kcp groups' exp to the DVE as an integer
# Schraudolph (one tensor_scalar f32->int16 producing bf16 bit patterns:
# bits = round(s*128/ln2 + 16256 - C); max rel err ~3%, which softmax
# normalization averages down to <3e-3 on the output). Head 1's DVE also
# carries the projection work, so it gets fewer tiles.
import math

SCH_SCALE = 128.0 / math.log(2.0)
SCH_BIAS = 16256.0 - 7.5
DVE_EXP_KCP = {0: (2, 5, 8, 11, 14), 1: (11, 13, 15)}


def _hoist_waits(nc):
    """The staged walrus build rejects instructions carrying more than one
    inline sync wait ("Too many sync wait commands"). Move every instruction's
    on_wait list into standalone EventSemaphore instructions immediately
    before it (same engine, same block) -- the encoding raw-bass wait_ge uses."""
    ctr = 0
    for bb in nc.main_func.blocks:
        out = []
        changed = False
        for ins in bb.instructions:
            si = getattr(ins, "sync_info", None)
            if si is not None and si.on_wait:
                for w in si.on_wait:
                    ctr += 1
                    out.append(
                        mybir.InstEventSemaphore(
                            name=f"hoistw-{ctr}",
                            opcode="EventSemaphore",
                            engine=ins.engine,
                            ins=[],
                            outs=[],
                            sync_info=mybir.SyncInfo(on_wait=[w], on_update=[]),
                        )
                    )
                ins.sync_info = mybir.SyncInfo(on_wait=[], on_update=si.on_update)
                changed = True
            out.append(ins)
        if changed:
            try:
                bb.instructions = out
            except Exception:
                bb.instructions.clear()
                bb.instructions.extend(out)
    return nc


def build(reps: int = 1):
    nc = bass.Bass()

    NCC = C // 128  # 6 contraction chunks over C

    # host-pretransposed layouts: [128, co, m] / [96, h, C] (contiguous
    # per-partition strips -> HWDGE descriptors)
    xT = nc.declare_dram_parameter("xT", [128, NCC, N], BF16, isOutput=False)
    wq = nc.declare_dram_parameter("wq", [128, NCC, HD], BF16, isOutput=False)
    wk = nc.declare_dram_parameter("wk", [128, NCC, HD], BF16, isOutput=False)
    wv = nc.declare_dram_parameter("wv", [128, NCC, HD], BF16, isOutput=False)
    wp = nc.declare_dram_parameter("wp", [96, HEADS_PER_CORE, C], BF16, isOutput=False)
    bias = nc.declare_dram_parameter("bias", [C], F32, isOutput=False)
    out_ext = nc.declare_dram_parameter("out", [N // 4, C], F32, isOutput=True)

    with tile.TileContext(nc) as tc:
        with (
            tc.tile_pool(name="ydram", bufs=5, space="DRAM") as ydram,
            tc.tile_pool(name="rsdram", bufs=5, space="DRAM") as rsdram,
            tc.tile_pool(name="const", bufs=1) as const,
            tc.tile_pool(name="ps", bufs=3, space="PSUM") as ps,
            tc.tile_pool(name="op", bufs=2, space="PSUM") as op,
            tc.tile_pool(name="pp", bufs=6) as pp,
            tc.tile_pool(name="misc", bufs=4) as misc,
            tc.tile_pool(name="yb", bufs=4) as ybp,
        ):
            for _rep in range(reps):
                # ---------------- constant loads ----------------
                # weights on the SP ring, xT chunks split across ACT+SP rings
                # so the first QKV matmul only gates on wq + xT chunk 0
                wq_sb = const.tile([128, NCC, HD], BF16)
                nc.scalar.dma_start(wq_sb, wq[:, :, :])
                wk_sb = const.tile([128, NCC, HD], BF16)
                wv_sb = const.tile([128, NCC, HD], BF16)
                wp_sb = const.tile([96, HEADS_PER_CORE, C], BF16)
                xT_sb = const.tile([128, NCC, N], BF16)

                def load_x(lo, hi):
                    for c in range(NCC):
                        eng = nc.scalar if c % 2 == 0 else nc.sync
                        eng.dma_start(
                            xT_sb[:, c, lo:hi], xT[:, c, lo:hi]
                        )

                # wq leads the scalar ring so the first q-unit gates only on
                # it + the six 0:512 xT slices; wk/wv follow the first slices
                load_x(0, 512)
                nc.sync.dma_start(wk_sb, wk[:, :, :])
                nc.sync.dma_start(wv_sb, wv[:, :, :])
                load_x(512, 1024)
                nc.sync.dma_start(wp_sb, wp[:, :, :])
                for nq in range(1, 4):
                    load_x(nq * 1024, (nq + 1) * 1024)
                # bias prefill: broadcast b_proj over all output rows; the
                # per-qr epilogue accumulates the RS result on top
                nc.gpsimd.dma_start(
                    out_ext[:, :],
                    bass.AP(
                        tensor=bias.ap().tensor, offset=0, ap=[[0, N // 4], [1, C]]
                    ),
                )

                # ---------------- QKV ----------------
                # q^T, k^T in [Dh, N] layout; v in [N, Dh] layout with an
                # appended ones column (softmax denominator row of oT).
                qT_sb = [const.tile([96, N], BF16, name=f"qT{h}") for h in range(2)]
                kT_sb = [const.tile([96, N], BF16, name=f"kT{h}") for h in range(2)]
                vp_sb = [const.tile([128, N_KC, 97], BF16, name=f"vp{h}") for h in range(2)]
                for h in range(2):
                    nc.vector.memset(vp_sb[h][:, :, 96:97], 1.0)
                # oT layout: [97(pad 128), n/128, 128] bf16 -- rows 0:96 = o,
                # row 96 = softmax denominator (bf16 copy of the PSUM row)
                oT_sb = [
                    const.tile([128, N // 128, 128], BF16, name=f"oT{h}")
                    for h in range(2)
                ]
                # reciprocal denominators, per q-partition, col = n-chunk
                rcp_sb = [const.tile([128, N // 128], F32, name=f"rcp{h}") for h in range(2)]
                ones_sb = const.tile([1, 1], BF16)
                nc.vector.memset(ones_sb, 1.0)

                def qk_units(h):
                    """q/k units for head h, ordered by xT n-quarter arrival."""
                    units = []

                    def qk_unit(w_sb, dst, n):
                        def emit():
                            acc = ps.tile([128, 512], F32, tag="ps")
                            for c in range(NCC):
                                nc.tensor.matmul(
                                    acc[:96, :],
                                    lhsT=w_sb[:, c, h * 96 : (h + 1) * 96],
                                    rhs=xT_sb[:, c, n * 512 : (n + 1) * 512],
                                    start=(c == 0),
                                    stop=(c == NCC - 1),
                                )
                            nc.vector.tensor_copy(
                                out=dst[:, n * 512 : (n + 1) * 512],
                                in_=acc[:96, :],
                            )

                        return emit

                    for n2 in range(N // 1024):
                        units.append(qk_unit(wq_sb, qT_sb[h], 2 * n2))
                        units.append(qk_unit(wq_sb, qT_sb[h], 2 * n2 + 1))
                        units.append(qk_unit(wk_sb, kT_sb[h], 2 * n2))
                        units.append(qk_unit(wk_sb, kT_sb[h], 2 * n2 + 1))
                    return units

                def v_units():
                    """Paired v units: both heads' v per ldweights (192-col
                    streams); unit n2 covers n-chunks 2*n2, 2*n2+1."""
                    units = []

                    def v_unit(n2):
                        def emit():
                            vacc = ps.tile([128, 1024], F32, tag="ps")
                            for half in range(2):
                                n = 2 * n2 + half
                                for c in range(NCC):
                                    nc.tensor.matmul(
                                        vacc[:, half * 512 : half * 512 + HD],
                                        lhsT=xT_sb[:, c, n * 128 : (n + 1) * 128],
                                        rhs=wv_sb[:, c, 0:HD],
                                        start=(c == 0),
                                        stop=(c == NCC - 1),
                                    )
                            for half in range(2):
                                for h in range(2):
                                    nc.vector.tensor_copy(
                                        out=vp_sb[h][:, 2 * n2 + half, 0:96],
                                        in_=vacc[
                                            :, half * 512 + h * 96 : half * 512 + (h + 1) * 96
                                        ],
                                    )

                        return emit

                    for n2 in range(N // 256):
                        units.append(v_unit(n2))
                    return units

                # ---------------- chunked output combine ----------------
                # RS chunks over qr ranges: big chunks early (amortize the
                # per-collective fixed cost), small chunks at the end (short
                # exposed tail). Each chunk has its own bounce tile so a
                # chunk's RS read never blocks the next chunk's proj writes.
                yb_tiles = {}  # chunk idx -> dram tile
                rs_tiles = {}
                RS_CHUNKS = [(0, 2), (2, 2), (4, 2), (6, 1), (7, 1)]
                QR_CHUNK = {}
                for ci, (a, nq) in enumerate(RS_CHUNKS):
                    for q in range(a, a + nq):
                        QR_CHUNK[q] = ci

                def emit_rs(ci):
                    a, nq = RS_CHUNKS[ci]
                    rs_out = rsdram.tile(
                        [nq * SH, C], BF16, tag="rsout", name=f"rso{ci}"
                    )
                    if _DEBUG_NO_RS:
                        nc.sync.dma_start(rs_out[:, :], yb_tiles[ci][0 : nq * SH, :])
                    else:
                        nc.gpsimd.collective_compute(
                            "ReduceScatter",
                            mybir.AluOpType.add,
                            replica_groups=GROUPS,
                            ins=[yb_tiles[ci][:, :].opt()],
                            outs=[rs_out.opt()],
                        )
                    rs_tiles[ci] = rs_out

                def emit_epilogue(ci):
                    # Pool queue: this read WAITS on the collective's output;
                    # on the in-order SP queue it would stall later y-writes
                    a, nq = RS_CHUNKS[ci]
                    rt = ybp.tile([128, nq, C], BF16, tag="rsb", bufs=3)
                    nc.gpsimd.dma_start(
                        rt, rs_tiles[ci].rearrange("(o p) m -> p o m", p=128)
                    )
                    # convert on the (idle) Pool engine: the whole epilogue
                    # chain waits on the collective, and on the in-order DVE
                    # queue that wait would block later exp/proj-scale work
                    rtf = ybp.tile([128, nq, C], F32, tag="rsf", bufs=3)
                    nc.gpsimd.tensor_copy(out=rtf, in_=rt)
                    nc.gpsimd.dma_start(
                        out_ext[a * SH : (a + nq) * SH, :].rearrange(
                            "(o p) m -> p o m", p=128
                        ),
                        rtf,
                        accum_op=mybir.AluOpType.add,
                    )

                def rcp_bc(h, nchunk):
                    a = rcp_sb[h][:, nchunk : nchunk + 1]
                    return bass.AP(
                        tensor=a.tensor,
                        offset=a.offset,
                        ap=list(a.ap[:-1]) + [[0, C]],
                    )

                y0_tiles = {}
                from functools import partial

                def oc_copy(h, qr, ot, dnrow):
                    # rows 0:96 = o, row 96 = denominator (bf16)
                    nc.vector.tensor_copy(
                        out=oT_sb[h][0:97, qr * 4 : (qr + 1) * 4, :],
                        in_=ot[0:97, :],
                    )
                    # denominator row: SBUF partition 96 -> partition 0
                    # (PE lhsT base must be 0/32/64)
                    nc.scalar.dma_start(
                        dnrow, oT_sb[h][96:97, qr * 4 : (qr + 1) * 4, :]
                    )

                def dn_recip(h, qr, dnrow):
                    # transpose the denominator row into per-partition
                    # form: 4 tiny K=1 matmuls, then one reciprocal
                    dn = ps.tile([128, 4], F32, tag="ps")
                    for j in range(4):
                        nc.tensor.matmul(
                            dn[:, j : j + 1],
                            lhsT=dnrow[0:1, j * 128 : (j + 1) * 128],
                            rhs=ones_sb[0:1, 0:1],
                            start=True,
                            stop=True,
                        )
                    nc.vector.reciprocal(rcp_sb[h][:, qr * 4 : (qr + 1) * 4], dn)

                def proj_half(qr, j, hh):
                    # y[nchunk] (+)= (oT_hh[:, nchunk]^T @ wp_hh) / denom_hh
                    nchunk = qr * 4 + j
                    ci = QR_CHUNK[qr]
                    a, nq = RS_CHUNKS[ci]
                    if hh == 0 and j == 0 and qr == a:
                        yb_tiles[ci] = ydram.tile(
                            [nq * QR, C], BF16, tag="yb", name=f"yb{ci}"
                        )
                    yp = ps.tile([128, 1024], F32, tag="ps")
                    for lo, hi in [(0, 512), (512, 768)]:
                        # start/stop per matmul: the two column ranges live in
                        # different PSUM banks (2KB zero regions), and each
                        # gets exactly one matmul
                        nc.tensor.matmul(
                            yp[:, lo:hi],
                            lhsT=oT_sb[hh][0:96, nchunk, :],
                            rhs=wp_sb[:96, hh, lo:hi],
                            start=True,
                            stop=True,
                        )
                    if hh == 0:
                        # qr7's four h0-halves live until the post-loop tail
                        # consumes them, overlapping qr6's transient pair
                        y0_sb = ybp.tile([128, C], BF16, tag="y0", bufs=10)
                        nc.vector.tensor_scalar(
                            out=y0_sb,
                            in0=yp[:, :C],
                            scalar1=rcp_sb[0][:, nchunk : nchunk + 1],
                            scalar2=None,
                            op0=mybir.AluOpType.mult,
                        )
                        y0_tiles[(qr, j)] = y0_sb
                    else:
                        y1_sb = ybp.tile([128, C], BF16, tag="y1", bufs=4)
                        nc.vector.tensor_scalar(
                            out=y1_sb,
                            in0=yp[:, :C],
                            scalar1=rcp_sb[1][:, nchunk : nchunk + 1],
                            scalar2=None,
                            op0=mybir.AluOpType.mult,
                        )
                        y_sb = ybp.tile([128, C], BF16, tag="y", bufs=4)
                        nc.vector.tensor_tensor(
                            y_sb, y0_tiles.pop((qr, j)), y1_sb,
                            mybir.AluOpType.add,
                        )
                        lo = (qr - a) * QR + j * 128
                        nc.sync.dma_start(yb_tiles[ci][lo : lo + 128, :], y_sb)

                def attention(h, unit_plan, carry_in=None):
                    """Flash attention for head h over all q-ranges; emits
                    closures from unit_plan[(qr, kcp)] between score groups.
                    Each q-range's finalize (oT copy / denominator transpose /
                    proj / RS) is deferred into the NEXT q-range's score
                    stream as staged closures."""
                    unit_plan = unit_plan or {}
                    stages = dict(carry_in or {})
                    for qr in range(N_QR):
                        # oT accumulator: rows 0:96 = o[d, q], row 96 = denom
                        ot = op.tile([128, 512], F32, tag="o")

                        def emit_pv(qr, kcp, p_t):
                            for kk in range(2):
                                kc = 2 * kcp + kk
                                nc.tensor.matmul(
                                    ot[0:97, :],
                                    lhsT=vp_sb[h][:, kc, 0:97],
                                    rhs=p_t[:, kk, :],
                                    start=(kcp == 0 and kk == 0),
                                    stop=(kcp == N_KC // 2 - 1 and kk == 1),
                                    skip_group_check=True,
                                )

                        # PV is software-pipelined one group behind the
                        # score/exp stream: the in-order PE queue would
                        # otherwise stall on exp(g) between QK(g) and PV(g).
                        pv_prev = None
                        for kcp in range(N_KC // 2):
                            sp = ps.tile([128, 2, 512], F32, tag="ps")
                            for kk in range(2):
                                kc = 2 * kcp + kk
                                nc.tensor.matmul(
                                    sp[:, kk, :],
                                    lhsT=kT_sb[h][:, kc * 128 : (kc + 1) * 128],
                                    rhs=qT_sb[h][:, qr * QR : (qr + 1) * QR],
                                    start=True,
                                    stop=True,
                                )
                            p_t = pp.tile([128, 2, 512], BF16, tag="p")
                            if kcp in DVE_EXP_KCP[h]:
                                nc.vector.tensor_scalar(
                                    out=p_t[:, :, :].bitcast(mybir.dt.int16),
                                    in0=sp,
                                    scalar1=SCH_SCALE,
                                    scalar2=SCH_BIAS,
                                    op0=mybir.AluOpType.mult,
                                    op1=mybir.AluOpType.add,
                                )
                            else:
                                nc.scalar.activation(
                                    p_t, sp, mybir.ActivationFunctionType.Exp
                                )
                            if pv_prev is not None:
                                emit_pv(qr, kcp - 1, pv_prev)
                            pv_prev = p_t
                            for u in stages.pop(kcp, ()):
                                u()
                            for u in unit_plan.pop((qr, kcp), ()):
                                u()
                        emit_pv(qr, N_KC // 2 - 1, pv_prev)

                        dnrow = misc.tile([1, 512], BF16, tag="dnr", bufs=4)
                        stages = {
                            0: [partial(oc_copy, h, qr, ot, dnrow)],
                            # stage 2 (not 1): the dn matmuls sit on the
                            # in-order PE queue, so give the oc-copy + dnrow
                            # DMA chain two score-groups of slack
                            2: [partial(dn_recip, h, qr, dnrow)],
                        }
                        if h == 1:
                            # qr 7's h0-half is injected into qr 7's own
                            # stream (oT_0/rcp_0 have long been final), so the
                            # post-loop tail only carries the h1-half + RS
                            for j in range(4):
                                if qr < N_QR - 1:
                                    stages.setdefault(3 + 2 * j, []).append(
                                        partial(proj_half, qr, j, 0)
                                    )
                                stages.setdefault(4 + 2 * j, []).append(
                                    partial(proj_half, qr, j, 1)
                                )
                            ci = QR_CHUNK[qr]
                            if qr == RS_CHUNKS[ci][0] + RS_CHUNKS[ci][1] - 1:
                                stages.setdefault(11, []).append(partial(emit_rs, ci))
                                stages.setdefault(12, []).append(
                                    partial(emit_epilogue, ci)
                                )
                    return stages

                # head-0 QKV emitted just-in-time inside head-0's own
                # attention sweep; paired v units serve both heads
                u0 = qk_units(0)
                q_un = [u0[4 * i + j] for i in range(4) for j in (0, 1)]
                k_un = [u0[4 * i + j] for i in range(4) for j in (2, 3)]
                v_un = v_units()
                plan = {}

                def put(qr, kcp, u):
                    plan.setdefault((qr, kcp), []).append(u)

                # v_un[n] feeds PV at kcp=n (PV runs one group late, so
                # stage n-1 suffices; v_un[0] rides at stage 0, off the
                # first-exp critical path); k-half n feeds scores at kcp=2n;
                # q-half n feeds q-range n
                for n in range(16):
                    put(0, max(n - 1, 0), v_un[n])
                for n in range(1, 8):
                    put(0, 2 * n - 2, k_un[n])
                for n in range(1, 8):
                    put(n - 1, 5, q_un[n])
                u1 = qk_units(1)
                if _INTERLEAVE_QKV1:
                    slots = [
                        (qr, kcp)
                        for qr in range(1, 8)
                        for kcp in (4, 7, 10, 13, 15)
                    ]
                    assert len(slots) >= len(u1)
                    for u, s in zip(u1, slots):
                        put(s[0], s[1], u)
                    u1 = []
                for u in (q_un[0], k_un[0], *u1):
                    u()
                # head 0's last finalize stages carry into head 1's stream;
                # head 1's qr-7 h0-proj rides inside head-1's own last qr
                plan1 = {}
                for j in range(4):
                    plan1[(N_QR - 1, 2 + 2 * j)] = [
                        partial(proj_half, N_QR - 1, j, 0)
                    ]
                carry = attention(0, plan)
                last = attention(1, plan1, carry_in=carry)
                for kcp in sorted(last):
                    for u in last[kcp]:
                        u()

    if _HOIST:
        _hoist_waits(nc)
    return nc


_NC_CACHE = None


def _get_nc():
    global _NC_CACHE
    if _NC_CACHE is None:
        _NC_CACHE = build()
    return _NC_CACHE


def make_in_maps(x, w_qkv, w_proj, b_proj):
    x = np.asarray(x, dtype=np.float32)
    w_qkv = np.asarray(w_qkv, dtype=np.float32).reshape(C, 3, H, DH)
    w_proj = np.asarray(w_proj, dtype=np.float32)
    b_proj = np.asarray(b_proj, dtype=np.float32)
    scale = DH ** -0.5
    NCC = C // 128

    def chunked(w):  # [C, M] -> [128, NCC, M] (partition-contiguous strips)
        return np.ascontiguousarray(
            w.reshape(NCC, 128, -1).transpose(1, 0, 2)
        ).astype(BF16_NP)

    xT_b = [
        chunked(np.ascontiguousarray(x[b].T)) for b in range(B)
    ]  # [128, NCC, N] each
    in_maps = []
    for i in range(N_CORES):
        b = i // 4
        h0 = HEADS_PER_CORE * (i % 4)
        sl = slice(h0, h0 + HEADS_PER_CORE)
        wq_i = chunked(w_qkv[:, 0, sl, :].reshape(C, HD) * scale)
        wk_i = chunked(w_qkv[:, 1, sl, :].reshape(C, HD))
        wv_i = chunked(w_qkv[:, 2, sl, :].reshape(C, HD))
        wp_i = np.ascontiguousarray(
            w_proj.reshape(H, DH, C)[sl].reshape(HEADS_PER_CORE, DH, C)
            .transpose(1, 0, 2)
        ).astype(BF16_NP)  # [96, 2, C]
        in_maps.append(
            {
                "xT": xT_b[b],
                "wq": wq_i,
                "wk": wk_i,
                "wv": wv_i,
                "wp": wp_i,
                "bias": b_proj,
            }
        )
    return in_maps


RS_CHUNKS_HOST = [(0, 2), (2, 2), (4, 2), (6, 1), (7, 1)]


def assemble(results):
    # rank r of batch-group b holds, per RS chunk (a, nq) covering global
    # rows [a*512, (a+nq)*512), the piece [a*512 + r*nq*128, ... + nq*128)
    # at out_ext rows [a*128, (a+nq)*128)
    out = np.empty((B, N, C), dtype=np.float32)
    for i in range(N_CORES):
        b, r = i // 4, i % 4
        shard = results[i]["out"]
        for a, nq in RS_CHUNKS_HOST:
            rows = nq * SH
            lo = a * QR + r * rows
            out[b, lo : lo + rows, :] = shard[a * SH : a * SH + rows]
    return out


def kernel(x, w_qkv, w_proj, b_proj):
    nc = _get_nc()
    in_maps = make_in_maps(x, w_qkv, w_proj, b_proj)
    res = run_bass_kernel_spmd(nc, in_maps, core_ids=list(range(N_CORES)))
    return assemble(res.results)


# revision 35
# speedup vs baseline: 1.0362x; 1.0362x over previous
"""Distributed multi-head attention kernel for trn2 (8 NeuronCores), v4.

Problem: B=2, N=4096, C=768, H=8 heads, Dh=96.
    qkv = x @ w_qkv ; per-head softmax(q k^T / sqrt(Dh)) v ; out @ w_proj + b_proj

Sharding (data parallel on B, tensor parallel on heads):
    core i -> batch b = i//4, heads (2*(i%4), 2*(i%4)+1)

v4 = v2's attention core + v3's scheduling/collective fixes:
  - PV in o[q-part, d] orientation (v2): 97-col streams, denominator in
    PSUM col 96 via the ones-column on v, per-PARTITION normalization
    (cheap DVE reciprocal + broadcast multiply). On HW the per-(kc,j)
    LDWEIGHTS prefetch under the streams, so this beats the oT[d,q]
    orientation's longer 512-col streams.
  - PV is software-pipelined one kcp group behind the score/exp stream:
    the in-order PE queue would otherwise stall on exp(g) between QK(g)
    and PV(g).
  - part of the exp work runs on the DVE as an integer Schraudolph
    (one tensor_scalar f32->int16 writing bf16 bit patterns:
    bits = round(s*128/ln2 + 16256 - 7.5), ~3% max err, averaged to
    <3e-3 output err by softmax normalization). This takes the ACT
    engine off the critical path of the score stream.
  - chunked ReduceScatter on per-chunk bounce tiles (no tile-level WAR
    between a chunk's RS read and the next chunk's proj writes), with
    each chunk's epilogue deferred until the NEXT chunk's RS dispatch
    so the Pool queue never stalls waiting on a collective.
  - weights are pre-transposed on host to [128, co, m] so every DMA
    descriptor is a contiguous per-partition strip (HWDGE, not SWDGE).
  - v-units compute both heads per ldweights (192-col streams).

Math notes: scores ~ N(0,1) after the Dh^-0.5 scale (folded into w_q on
host), so softmax max-subtraction is skipped (exp < ~1e3, safe in f32).
Compute dtype bf16 on the PE (f32 PSUM accumulation); RS payload bf16.
"""

import math

import numpy as np
import ml_dtypes

import concourse.bass as bass
import concourse.tile as tile
from concourse import mybir
from concourse.bass_utils import run_bass_kernel_spmd

# ---------------- problem constants (hardcoded per spec) ----------------
B, N, C, H, DH = 2, 4096, 768, 8, 96
HEADS_PER_CORE = 2
HD = HEADS_PER_CORE * DH  # 192
N_CORES = 8
GROUPS = [[0, 1, 2, 3], [4, 5, 6, 7]]
QR = 512  # query rows per o-accumulation group (one PSUM bank of [128, 4, 97])
N_QR = N // QR  # 8
KC = 128  # key chunk (contraction tile for PV)
N_KC = N // KC  # 32
SH = QR // 4  # rows per rank per qr (128)

F32 = mybir.dt.float32
BF16 = mybir.dt.bfloat16
BF16_NP = ml_dtypes.bfloat16

_HOIST = True  # hoist inline waits (required for the walrus build; off for CoreSim)
_DEBUG_NO_RS = False  # replace ReduceScatter with a local copy (debug only)
_INTERLEAVE_QKV1 = True  # interleave head-1 QKV into head-0 attention

# exp offload: integer-Schraudolph exp on the DVE for some kcp groups
SCH_SCALE = 128.0 / math.log(2.0)
SCH_BIAS = 16256.0 - 7.5
DVE_EXP_KCP = {0: (2, 4, 7, 9, 12, 14), 1: (3, 9, 13, 15)}

# ReduceScatter chunks over qr ranges: (first_qr, n_qr). Big chunks early
# (amortize the ~20us per-collective fixed cost + cross-core rendezvous),
# small chunks last (short exposed tail).
RS_CHUNKS = [(0, 3), (3, 3), (6, 1), (7, 1)]


def _hoist_waits(nc):
    """The staged walrus build rejects instructions carrying more than one
    inline sync wait ("Too many sync wait commands"). Move every instruction's
    on_wait list into standalone EventSemaphore instructions immediately
    before it (same engine, same block) -- the encoding raw-bass wait_ge uses."""
    ctr = 0
    for bb in nc.main_func.blocks:
        out = []
        changed = False
        for ins in bb.instructions:
            si = getattr(ins, "sync_info", None)
            if si is not None and si.on_wait:
                for w in si.on_wait:
                    ctr += 1
                    out.append(
                        mybir.InstEventSemaphore(
                            name=f"hoistw-{ctr}",
                            opcode="EventSemaphore",
                            engine=ins.engine,
                            ins=[],
                            outs=[],
                            sync_info=mybir.SyncInfo(on_wait=[w], on_update=[]),
                        )
                    )
                ins.sync_info = mybir.SyncInfo(on_wait=[], on_update=si.on_update)
                changed = True
            out.append(ins)
        if changed:
            try:
                bb.instructions = out
            except Exception:
                bb.instructions.clear()
                bb.instructions.extend(out)
    return nc


def build(reps: int = 1):
    nc = bass.Bass()

    NCC = C // 128  # 6 contraction chunks over C

    # host-pretransposed layouts: [128, co, m] / [96, h, C] (contiguous
    # per-partition strips -> HWDGE descriptors)
    xT = nc.declare_dram_parameter("xT", [128, NCC, N], BF16, isOutput=False)
    wq = nc.declare_dram_parameter("wq", [128, NCC, HD], BF16, isOutput=False)
    wk = nc.declare_dram_parameter("wk", [128, NCC, HD], BF16, isOutput=False)
    wv = nc.declare_dram_parameter("wv", [128, NCC, HD], BF16, isOutput=False)
    wp = nc.declare_dram_parameter("wp", [96, HEADS_PER_CORE, C], BF16, isOutput=False)
    bias = nc.declare_dram_parameter("bias", [C], F32, isOutput=False)
    out_ext = nc.declare_dram_parameter("out", [N // 4, C], F32, isOutput=True)

    QR_CHUNK = {}
    for ci, (a, nq) in enumerate(RS_CHUNKS):
        for q in range(a, a + nq):
            QR_CHUNK[q] = ci

    with tile.TileContext(nc) as tc:
        with (
            tc.tile_pool(name="ydram", bufs=len(RS_CHUNKS) + 1, space="DRAM") as ydram,
            tc.tile_pool(name="rsdram", bufs=len(RS_CHUNKS) + 1, space="DRAM") as rsdram,
            tc.tile_pool(name="const", bufs=1) as const,
            tc.tile_pool(name="ps", bufs=3, space="PSUM") as ps,
            tc.tile_pool(name="op", bufs=2, space="PSUM") as op,
            tc.tile_pool(name="pp", bufs=6) as pp,
            tc.tile_pool(name="onp", bufs=3) as onp,
            tc.tile_pool(name="misc", bufs=4) as misc,
            tc.tile_pool(name="yb", bufs=4) as ybp,
        ):
            for _rep in range(reps):
                # ---------------- constant loads ----------------
                wq_sb = const.tile([128, NCC, HD], BF16)
                nc.scalar.dma_start(wq_sb, wq[:, :, :])
                wk_sb = const.tile([128, NCC, HD], BF16)
                wv_sb = const.tile([128, NCC, HD], BF16)
                wp_sb = const.tile([96, HEADS_PER_CORE, C], BF16)
                xT_sb = const.tile([128, NCC, N], BF16)

                def load_x(lo, hi):
                    for c in range(NCC):
                        eng = nc.scalar if c % 2 == 0 else nc.sync
                        eng.dma_start(xT_sb[:, c, lo:hi], xT[:, c, lo:hi])

                # wq leads the scalar ring so the first q-unit gates only on
                # it + the six 0:512 xT slices; wk/wv follow the first slices
                load_x(0, 512)
                nc.sync.dma_start(wk_sb, wk[:, :, :])
                nc.sync.dma_start(wv_sb, wv[:, :, :])
                load_x(512, 1024)
                nc.sync.dma_start(wp_sb, wp[:, :, :])
                for nq4 in range(1, 4):
                    load_x(nq4 * 1024, (nq4 + 1) * 1024)
                # bias prefill: broadcast b_proj over all output rows; the
                # per-chunk epilogue accumulates the RS result on top
                nc.gpsimd.dma_start(
                    out_ext[:, :],
                    bass.AP(
                        tensor=bias.ap().tensor, offset=0, ap=[[0, N // 4], [1, C]]
                    ),
                )

                # ---------------- QKV ----------------
                # q^T, k^T in [Dh, N] layout; v in [N, Dh] layout with an
                # appended ones column (softmax denominator).
                qT_sb = [const.tile([96, N], BF16, name=f"qT{h}") for h in range(2)]
                kT_sb = [const.tile([96, N], BF16, name=f"kT{h}") for h in range(2)]
                vp_sb = [const.tile([128, N_KC, 97], BF16, name=f"vp{h}") for h in range(2)]
                for h in range(2):
                    nc.vector.memset(vp_sb[h][:, :, 96:97], 1.0)
                # normalized-oT layout for the projection: [128, 32, 128] ==
                # [Dh(pad), n/128, 128] bf16 (d padded 96->128 so the XBAR DMA
                # transpose tiles align; proj reads partitions 0:96 only)
                onT_sb = [
                    const.tile([128, N // 128, 128], BF16, name=f"onT{h}")
                    for h in range(2)
                ]

                def qk_units(h):
                    """q/k units for head h, ordered by xT n-quarter arrival."""
                    units = []

                    def qk_unit(w_sb, dst, n):
                        def emit():
                            acc = ps.tile([128, 512], F32, tag="ps")
                            for c in range(NCC):
                                nc.tensor.matmul(
                                    acc[:96, :],
                                    lhsT=w_sb[:, c, h * 96 : (h + 1) * 96],
                                    rhs=xT_sb[:, c, n * 512 : (n + 1) * 512],
                                    start=(c == 0),
                                    stop=(c == NCC - 1),
                                )
                            nc.vector.tensor_copy(
                                out=dst[:, n * 512 : (n + 1) * 512],
                                in_=acc[:96, :],
                            )

                        return emit

                    for n2 in range(N // 1024):
                        units.append(qk_unit(wq_sb, qT_sb[h], 2 * n2))
                        units.append(qk_unit(wq_sb, qT_sb[h], 2 * n2 + 1))
                        units.append(qk_unit(wk_sb, kT_sb[h], 2 * n2))
                        units.append(qk_unit(wk_sb, kT_sb[h], 2 * n2 + 1))
                    return units

                def v_units():
                    """Paired v units: both heads' v per ldweights (192-col
                    streams); unit n2 covers n-chunks 2*n2, 2*n2+1."""
                    units = []

                    def v_unit(n2):
                        def emit():
                            vacc = ps.tile([128, 1024], F32, tag="ps")
                            for half in range(2):
                                n = 2 * n2 + half
                                for c in range(NCC):
                                    nc.tensor.matmul(
                                        vacc[:, half * 512 : half * 512 + HD],
                                        lhsT=xT_sb[:, c, n * 128 : (n + 1) * 128],
                                        rhs=wv_sb[:, c, 0:HD],
                                        start=(c == 0),
                                        stop=(c == NCC - 1),
                                    )
                            for half in range(2):
                                for h in range(2):
                                    nc.vector.tensor_copy(
                                        out=vp_sb[h][:, 2 * n2 + half, 0:96],
                                        in_=vacc[
                                            :, half * 512 + h * 96 : half * 512 + (h + 1) * 96
                                        ],
                                    )

                        return emit

                    for n2 in range(N // 256):
                        units.append(v_unit(n2))
                    return units

                # ---------------- chunked output combine ----------------
                yb_tiles = {}
                rs_tiles = {}

                def emit_rs(ci):
                    a, nq = RS_CHUNKS[ci]
                    rs_out = rsdram.tile(
                        [nq * SH, C], BF16, tag="rsout", name=f"rso{ci}"
                    )
                    if _DEBUG_NO_RS:
                        nc.sync.dma_start(rs_out[:, :], yb_tiles[ci][0 : nq * SH, :])
                    else:
                        nc.gpsimd.collective_compute(
                            "ReduceScatter",
                            mybir.AluOpType.add,
                            replica_groups=GROUPS,
                            ins=[yb_tiles[ci][:, :].opt()],
                            outs=[rs_out.opt()],
                        )
                    rs_tiles[ci] = rs_out

                def emit_epilogue(ci, final=False):
                    # Deferred one chunk: by the time this is dispatched the
                    # collective is long done, so the Pool-queue waits are
                    # pre-satisfied and never delay the next RS dispatch.
                    a, nq = RS_CHUNKS[ci]
                    rt = ybp.tile([128, nq, C], BF16, tag="rsb", bufs=2)
                    nc.gpsimd.dma_start(
                        rt, rs_tiles[ci].rearrange("(o p) m -> p o m", p=128)
                    )
                    rtf = ybp.tile([128, nq, C], F32, tag="rsf", bufs=2)
                    if final:
                        # tail: DVE is idle and ~2x faster than the Q7 copy
                        nc.vector.tensor_copy(out=rtf, in_=rt)
                    else:
                        # mid-stream: Pool engine, off the DVE exp path
                        nc.gpsimd.tensor_copy(out=rtf, in_=rt)
                    nc.gpsimd.dma_start(
                        out_ext[a * SH : (a + nq) * SH, :].rearrange(
                            "(o p) m -> p o m", p=128
                        ),
                        rtf,
                        accum_op=mybir.AluOpType.add,
                    )

                from functools import partial

                def norm_transpose(h, qr, o4):
                    # normalize per-partition: on = o[:,:96] / o[:,96]
                    # (d padded to 128 for XBAR alignment; pad zeroed)
                    on_qr = onp.tile([128, 4, 128], BF16, tag="on")
                    nc.vector.memset(on_qr[:, :, 96:128], 0.0)
                    r4 = misc.tile([128, 4, 1], F32, tag="r")
                    nc.vector.reciprocal(r4, o4[:, :, 96:97])
                    r4_ap = r4[:, :, :]
                    r4_bc = bass.AP(
                        tensor=r4_ap.tensor,
                        offset=r4_ap.offset,
                        ap=list(r4_ap.ap[:-1]) + [[0, 96]],
                    )
                    nc.vector.tensor_tensor(
                        on_qr[:, :, 0:96],
                        o4[:, :, 0:96],
                        r4_bc,
                        mybir.AluOpType.mult,
                    )
                    # oT via the DMA XBAR transpose (SP queue; dispatch is
                    # async from the transfer)
                    nc.sync.dma_start_transpose(
                        onT_sb[h][:, qr * 4 : (qr + 1) * 4, :], on_qr
                    )

                def proj_chunk(qr, j):
                    # projection: y[n,:] = sum_h onT_h[:, n]^T @ wp_h
                    nchunk = qr * 4 + j
                    ci = QR_CHUNK[qr]
                    a, nq = RS_CHUNKS[ci]
                    if j == 0 and qr == a:
                        yb_tiles[ci] = ydram.tile(
                            [nq * QR, C], BF16, tag="yb", name=f"yb{ci}"
                        )
                    yp = ps.tile([128, 1024], F32, tag="ps")
                    for hh in range(2):
                        for lo, hi in [(0, 512), (512, 768)]:
                            nc.tensor.matmul(
                                yp[:, lo:hi],
                                lhsT=onT_sb[hh][0:96, nchunk, :],
                                rhs=wp_sb[:96, hh, lo:hi],
                                start=(hh == 0),
                                stop=(hh == 1),
                            )
                    y_sb = ybp.tile([128, C], BF16, tag="y")
                    nc.vector.tensor_copy(out=y_sb, in_=yp[:, :C])
                    lo = (qr - a) * QR + j * 128
                    nc.sync.dma_start(yb_tiles[ci][lo : lo + 128, :], y_sb)

                def attention(h, unit_plan, carry_in=None):
                    """Flash attention for head h over all q-ranges; emits
                    closures from unit_plan[(qr, kcp)] between score groups.
                    Each q-range's finalize (normalize/transpose/proj/RS) is
                    deferred into the NEXT q-range's score stream."""
                    unit_plan = unit_plan or {}
                    stages = dict(carry_in or {})
                    for qr in range(N_QR):
                        # all 4 q128-chunk accumulators share ONE psum bank;
                        # the first matmul's start=True zeroes the whole 2KB
                        # zero-region, later j's first matmuls ride on it
                        o4 = op.tile([128, 4, 97], F32, tag="o")

                        def emit_pv(kcp, p_t):
                            for kk in range(2):
                                kc = 2 * kcp + kk
                                for j in range(4):
                                    nc.tensor.matmul(
                                        o4[:, j, 0:97],
                                        lhsT=p_t[:, kk, j * 128 : (j + 1) * 128],
                                        rhs=vp_sb[h][:, kc, :],
                                        start=(kcp == 0 and kk == 0 and j == 0),
                                        stop=(kcp == N_KC // 2 - 1 and kk == 1),
                                        skip_group_check=True,
                                    )

                        # PV runs one kcp group behind the score/exp stream:
                        # the in-order PE queue would otherwise stall on
                        # exp(g) between QK(g) and PV(g)
                        pv_prev = None
                        for kcp in range(N_KC // 2):
                            sp = ps.tile([128, 2, 512], F32, tag="ps")
                            for kk in range(2):
                                kc = 2 * kcp + kk
                                nc.tensor.matmul(
                                    sp[:, kk, :],
                                    lhsT=kT_sb[h][:, kc * 128 : (kc + 1) * 128],
                                    rhs=qT_sb[h][:, qr * QR : (qr + 1) * QR],
                                    start=True,
                                    stop=True,
                                )
                            p_t = pp.tile([128, 2, 512], BF16, tag="p")
                            if kcp in DVE_EXP_KCP[h]:
                                nc.vector.tensor_scalar(
                                    out=p_t[:, :, :].bitcast(mybir.dt.int16),
                                    in0=sp,
                                    scalar1=SCH_SCALE,
                                    scalar2=SCH_BIAS,
                                    op0=mybir.AluOpType.mult,
                                    op1=mybir.AluOpType.add,
                                )
                            else:
                                nc.scalar.activation(
                                    p_t, sp, mybir.ActivationFunctionType.Exp
                                )
                            if pv_prev is not None:
                                emit_pv(kcp - 1, pv_prev)
                            pv_prev = p_t
                            for u in stages.pop(kcp, ()):
                                u()
                            for u in unit_plan.pop((qr, kcp), ()):
                                u()
                        emit_pv(N_KC // 2 - 1, pv_prev)

                        stages = {0: [partial(norm_transpose, h, qr, o4)]}
                        if h == 1:
                            for j in range(4):
                                stages.setdefault(2 + 2 * j, []).append(
                                    partial(proj_chunk, qr, j)
                                )
                            ci = QR_CHUNK[qr]
                            if qr == RS_CHUNKS[ci][0] + RS_CHUNKS[ci][1] - 1:
                                stages.setdefault(11, []).append(partial(emit_rs, ci))
                                # previous chunk's epilogue: its RS is done by
                                # now, so the waits don't stall the Pool queue
                                if ci > 0:
                                    stages.setdefault(13, []).append(
                                        partial(emit_epilogue, ci - 1)
                                    )
                                if ci == len(RS_CHUNKS) - 1:
                                    stages.setdefault(14, []).append(
                                        partial(emit_epilogue, ci, True)
                                    )
                    return stages

                # head-0 QKV emitted just-in-time inside head-0's own
                # attention sweep; paired v units serve both heads.
                # v_un[n] feeds PV at kcp=n (PV runs one group late, so
                # stage n-1 suffices; v_un[0] rides at stage 0, off the
                # first-exp critical path); k-half n feeds scores at kcp=2n;
                # q-half n feeds q-range n
                u0 = qk_units(0)
                q_un = [u0[4 * i + j] for i in range(4) for j in (0, 1)]
                k_un = [u0[4 * i + j] for i in range(4) for j in (2, 3)]
                v_un = v_units()
                plan = {}

                def put(qr, kcp, u):
                    plan.setdefault((qr, kcp), []).append(u)

                for n in range(16):
                    put(0, max(n - 1, 0), v_un[n])
                for n in range(1, 8):
                    put(0, 2 * n - 2, k_un[n])
                for n in range(1, 8):
                    put(n - 1, 5, q_un[n])
                u1 = qk_units(1)
                if _INTERLEAVE_QKV1:
                    slots = [
                        (qr, kcp)
                        for qr in range(1, 8)
                        for kcp in (4, 7, 10, 13, 15)
                    ]
                    assert len(slots) >= len(u1)
                    for u, s in zip(u1, slots):
                        put(s[0], s[1], u)
                    u1 = []
                for u in (q_un[0], k_un[0], *u1):
                    u()
                # head 0's last finalize stages carry into head 1's stream;
                # head 1's last stages (norm + proj tail + final RS +
                # epilogues) flush after the loop
                carry = attention(0, plan)
                last = attention(1, None, carry_in=carry)
                for kcp in sorted(last):
                    for u in last[kcp]:
                        u()

    if _HOIST:
        _hoist_waits(nc)
    return nc


_NC_CACHE = None


def _get_nc():
    global _NC_CACHE
    if _NC_CACHE is None:
        _NC_CACHE = build()
    return _NC_CACHE


def make_in_maps(x, w_qkv, w_proj, b_proj):
    x = np.asarray(x, dtype=np.float32)
    w_qkv = np.asarray(w_qkv, dtype=np.float32).reshape(C, 3, H, DH)
    w_proj = np.asarray(w_proj, dtype=np.float32)
    b_proj = np.asarray(b_proj, dtype=np.float32)
    scale = DH ** -0.5
    NCC = C // 128

    def chunked(w):  # [C, M] -> [128, NCC, M] (partition-contiguous strips)
        return np.ascontiguousarray(
            w.reshape(NCC, 128, -1).transpose(1, 0, 2)
        ).astype(BF16_NP)

    xT_b = [chunked(np.ascontiguousarray(x[b].T)) for b in range(B)]
    in_maps = []
    for i in range(N_CORES):
        b = i // 4
        h0 = HEADS_PER_CORE * (i % 4)
        sl = slice(h0, h0 + HEADS_PER_CORE)
        wq_i = chunked(w_qkv[:, 0, sl, :].reshape(C, HD) * scale)
        wk_i = chunked(w_qkv[:, 1, sl, :].reshape(C, HD))
        wv_i = chunked(w_qkv[:, 2, sl, :].reshape(C, HD))
        wp_i = np.ascontiguousarray(
            w_proj.reshape(H, DH, C)[sl].reshape(HEADS_PER_CORE, DH, C)
            .transpose(1, 0, 2)
        ).astype(BF16_NP)  # [96, 2, C]
        in_maps.append(
            {
                "xT": xT_b[b],
                "wq": wq_i,
                "wk": wk_i,
                "wv": wv_i,
                "wp": wp_i,
                "bias": b_proj,
            }
        )
    return in_maps


def assemble(results):
    # rank r of batch-group b holds, per RS chunk (a, nq) covering global
    # rows [a*512, (a+nq)*512), the piece [a*512 + r*nq*128, ... + nq*128)
    # at out_ext rows [a*128, (a+nq)*128)
    out = np.empty((B, N, C), dtype=np.float32)
    for i in range(N_CORES):
        b, r = i // 4, i % 4
        shard = results[i]["out"]
        for a, nq in RS_CHUNKS:
            rows = nq * SH
            lo = a * QR + r * rows
            out[b, lo : lo + rows, :] = shard[a * SH : a * SH + rows]
    return out


def kernel(x, w_qkv, w_proj, b_proj):
    nc = _get_nc()
    in_maps = make_in_maps(x, w_qkv, w_proj, b_proj)
    res = run_bass_kernel_spmd(nc, in_maps, core_ids=list(range(N_CORES)))
    return assemble(res.results)


# revision 37
# speedup vs baseline: 1.0938x; 1.0556x over previous
"""Distributed multi-head attention kernel for trn2 (8 NeuronCores), v4.

Problem: B=2, N=4096, C=768, H=8 heads, Dh=96.
    qkv = x @ w_qkv ; per-head softmax(q k^T / sqrt(Dh)) v ; out @ w_proj + b_proj

Sharding (data parallel on B, tensor parallel on heads):
    core i -> batch b = i//4, heads (2*(i%4), 2*(i%4)+1)

v4 = v2's attention core + v3's scheduling/collective fixes:
  - PV in o[q-part, d] orientation (v2): 97-col streams, denominator in
    PSUM col 96 via the ones-column on v, per-PARTITION normalization
    (cheap DVE reciprocal + broadcast multiply). On HW the per-(kc,j)
    LDWEIGHTS prefetch under the streams, so this beats the oT[d,q]
    orientation's longer 512-col streams.
  - PV is software-pipelined one kcp group behind the score/exp stream:
    the in-order PE queue would otherwise stall on exp(g) between QK(g)
    and PV(g).
  - part of the exp work runs on the DVE as an integer Schraudolph
    (one tensor_scalar f32->int16 writing bf16 bit patterns:
    bits = round(s*128/ln2 + 16256 - 7.5), ~3% max err, averaged to
    <3e-3 output err by softmax normalization). This takes the ACT
    engine off the critical path of the score stream.
  - chunked ReduceScatter on per-chunk bounce tiles (no tile-level WAR
    between a chunk's RS read and the next chunk's proj writes), with
    each chunk's epilogue deferred until the NEXT chunk's RS dispatch
    so the Pool queue never stalls waiting on a collective.
  - weights are pre-transposed on host to [128, co, m] so every DMA
    descriptor is a contiguous per-partition strip (HWDGE, not SWDGE).
  - v-units compute both heads per ldweights (192-col streams).

Math notes: scores ~ N(0,1) after the Dh^-0.5 scale (folded into w_q on
host), so softmax max-subtraction is skipped (exp < ~1e3, safe in f32).
Compute dtype bf16 on the PE (f32 PSUM accumulation); RS payload bf16.
"""

import math

import numpy as np
import ml_dtypes

import concourse.bass as bass
import concourse.tile as tile
from concourse import mybir
from concourse.bass_utils import run_bass_kernel_spmd

# ---------------- problem constants (hardcoded per spec) ----------------
B, N, C, H, DH = 2, 4096, 768, 8, 96
HEADS_PER_CORE = 2
HD = HEADS_PER_CORE * DH  # 192
N_CORES = 8
GROUPS = [[0, 1, 2, 3], [4, 5, 6, 7]]
QR = 512  # query rows per o-accumulation group (one PSUM bank of [128, 4, 97])
N_QR = N // QR  # 8
KC = 128  # key chunk (contraction tile for PV)
N_KC = N // KC  # 32
SH = QR // 4  # rows per rank per qr (128)

F32 = mybir.dt.float32
BF16 = mybir.dt.bfloat16
BF16_NP = ml_dtypes.bfloat16

_HOIST = True  # hoist inline waits (required for the walrus build; off for CoreSim)
_DEBUG_NO_RS = False  # replace ReduceScatter with a local copy (debug only)
_INTERLEAVE_QKV1 = True  # interleave head-1 QKV into head-0 attention

# exp offload: integer-Schraudolph exp on the DVE for some kcp groups
SCH_SCALE = 128.0 / math.log(2.0)
SCH_BIAS = 16256.0 - 7.5
DVE_EXP_KCP = {0: (2, 4, 7, 9, 12, 14), 1: (3, 9, 13, 15)}

# ReduceScatter chunks over qr ranges: (first_qr, n_qr). Big chunks early
# (amortize the ~20us per-collective fixed cost + cross-core rendezvous),
# small chunks last (short exposed tail).
RS_CHUNKS = [(0, 3), (3, 3), (6, 1), (7, 1)]


def _hoist_waits(nc):
    """The staged walrus build rejects instructions carrying more than one
    inline sync wait ("Too many sync wait commands"). Move every instruction's
    on_wait list into standalone EventSemaphore instructions immediately
    before it (same engine, same block) -- the encoding raw-bass wait_ge uses."""
    ctr = 0
    for bb in nc.main_func.blocks:
        out = []
        changed = False
        for ins in bb.instructions:
            si = getattr(ins, "sync_info", None)
            if si is not None and si.on_wait:
                for w in si.on_wait:
                    ctr += 1
                    out.append(
                        mybir.InstEventSemaphore(
                            name=f"hoistw-{ctr}",
                            opcode="EventSemaphore",
                            engine=ins.engine,
                            ins=[],
                            outs=[],
                            sync_info=mybir.SyncInfo(on_wait=[w], on_update=[]),
                        )
                    )
                ins.sync_info = mybir.SyncInfo(on_wait=[], on_update=si.on_update)
                changed = True
            out.append(ins)
        if changed:
            try:
                bb.instructions = out
            except Exception:
                bb.instructions.clear()
                bb.instructions.extend(out)
    return nc


def build(reps: int = 1):
    nc = bass.Bass()

    NCC = C // 128  # 6 contraction chunks over C

    # host-pretransposed layouts: [128, co, m] / [96, h, C] (contiguous
    # per-partition strips -> HWDGE descriptors)
    xT = nc.declare_dram_parameter("xT", [128, NCC, N], BF16, isOutput=False)
    wq = nc.declare_dram_parameter("wq", [128, NCC, HD], BF16, isOutput=False)
    wk = nc.declare_dram_parameter("wk", [128, NCC, HD], BF16, isOutput=False)
    wv = nc.declare_dram_parameter("wv", [128, NCC, HD], BF16, isOutput=False)
    wp = nc.declare_dram_parameter("wp", [96, HEADS_PER_CORE, C], BF16, isOutput=False)
    bias = nc.declare_dram_parameter("bias", [C], F32, isOutput=False)
    out_ext = nc.declare_dram_parameter("out", [N // 4, C], F32, isOutput=True)

    QR_CHUNK = {}
    for ci, (a, nq) in enumerate(RS_CHUNKS):
        for q in range(a, a + nq):
            QR_CHUNK[q] = ci

    with tile.TileContext(nc) as tc:
        with (
            tc.tile_pool(name="ydram", bufs=len(RS_CHUNKS) + 1, space="DRAM") as ydram,
            tc.tile_pool(name="rsdram", bufs=len(RS_CHUNKS) + 1, space="DRAM") as rsdram,
            tc.tile_pool(name="const", bufs=1) as const,
            tc.tile_pool(name="ps", bufs=3, space="PSUM") as ps,
            tc.tile_pool(name="op", bufs=2, space="PSUM") as op,
            tc.tile_pool(name="pp", bufs=6) as pp,
            tc.tile_pool(name="onp", bufs=3) as onp,
            tc.tile_pool(name="misc", bufs=4) as misc,
            tc.tile_pool(name="yb", bufs=4) as ybp,
        ):
            for _rep in range(reps):
                # ---------------- constant loads ----------------
                wq_sb = const.tile([128, NCC, HD], BF16)
                nc.scalar.dma_start(wq_sb, wq[:, :, :])
                wk_sb = const.tile([128, NCC, HD], BF16)
                wv_sb = const.tile([128, NCC, HD], BF16)
                wp_sb = const.tile([96, HEADS_PER_CORE, C], BF16)
                xT_sb = const.tile([128, NCC, N], BF16)

                # three dispatch rings so the startup loads overlap across
                # DMA queues (the pool ring is idle during the ramp)
                _x_rings = (nc.scalar, nc.sync, nc.gpsimd)

                def load_x(lo, hi):
                    for c in range(NCC):
                        _x_rings[c % 3].dma_start(
                            xT_sb[:, c, lo:hi], xT[:, c, lo:hi]
                        )

                # wq leads the scalar ring so the first q-unit gates only on
                # it + the six 0:512 xT slices; wk/wv follow the first slices
                load_x(0, 512)
                nc.sync.dma_start(wk_sb, wk[:, :, :])
                nc.sync.dma_start(wv_sb, wv[:, :, :])
                load_x(512, 1024)
                nc.sync.dma_start(wp_sb, wp[:, :, :])
                for nq4 in range(1, 4):
                    load_x(nq4 * 1024, (nq4 + 1) * 1024)
                # bias prefill: broadcast b_proj over all output rows; the
                # per-chunk epilogue accumulates the RS result on top
                nc.gpsimd.dma_start(
                    out_ext[:, :],
                    bass.AP(
                        tensor=bias.ap().tensor, offset=0, ap=[[0, N // 4], [1, C]]
                    ),
                )

                # ---------------- QKV ----------------
                # q^T, k^T in [Dh, N] layout; v in [N, Dh] layout with an
                # appended ones column (softmax denominator).
                qT_sb = [const.tile([96, N], BF16, name=f"qT{h}") for h in range(2)]
                kT_sb = [const.tile([96, N], BF16, name=f"kT{h}") for h in range(2)]
                vp_sb = [const.tile([128, N_KC, 97], BF16, name=f"vp{h}") for h in range(2)]
                for h in range(2):
                    nc.vector.memset(vp_sb[h][:, :, 96:97], 1.0)
                # normalized-oT layout for the projection: [128, 32, 128] ==
                # [Dh(pad), n/128, 128] bf16 (d padded 96->128 so the XBAR DMA
                # transpose tiles align; proj reads partitions 0:96 only)
                onT_sb = [
                    const.tile([128, N // 128, 128], BF16, name=f"onT{h}")
                    for h in range(2)
                ]

                def qk_units(h):
                    """q/k units for head h, ordered by xT n-quarter arrival."""
                    units = []

                    def qk_unit(w_sb, dst, n):
                        def emit():
                            acc = ps.tile([128, 512], F32, tag="ps")
                            for c in range(NCC):
                                nc.tensor.matmul(
                                    acc[:96, :],
                                    lhsT=w_sb[:, c, h * 96 : (h + 1) * 96],
                                    rhs=xT_sb[:, c, n * 512 : (n + 1) * 512],
                                    start=(c == 0),
                                    stop=(c == NCC - 1),
                                )
                            nc.vector.tensor_copy(
                                out=dst[:, n * 512 : (n + 1) * 512],
                                in_=acc[:96, :],
                            )

                        return emit

                    for n2 in range(N // 1024):
                        units.append(qk_unit(wq_sb, qT_sb[h], 2 * n2))
                        units.append(qk_unit(wq_sb, qT_sb[h], 2 * n2 + 1))
                        units.append(qk_unit(wk_sb, kT_sb[h], 2 * n2))
                        units.append(qk_unit(wk_sb, kT_sb[h], 2 * n2 + 1))
                    return units

                def v_units():
                    """Paired v units: both heads' v per ldweights (192-col
                    streams); unit n2 covers n-chunks 2*n2, 2*n2+1."""
                    units = []

                    def v_unit(n2):
                        def emit():
                            vacc = ps.tile([128, 1024], F32, tag="ps")
                            for half in range(2):
                                n = 2 * n2 + half
                                for c in range(NCC):
                                    nc.tensor.matmul(
                                        vacc[:, half * 512 : half * 512 + HD],
                                        lhsT=xT_sb[:, c, n * 128 : (n + 1) * 128],
                                        rhs=wv_sb[:, c, 0:HD],
                                        start=(c == 0),
                                        stop=(c == NCC - 1),
                                    )
                            for half in range(2):
                                for h in range(2):
                                    nc.vector.tensor_copy(
                                        out=vp_sb[h][:, 2 * n2 + half, 0:96],
                                        in_=vacc[
                                            :, half * 512 + h * 96 : half * 512 + (h + 1) * 96
                                        ],
                                    )

                        return emit

                    for n2 in range(N // 256):
                        units.append(v_unit(n2))
                    return units

                # ---------------- chunked output combine ----------------
                yb_tiles = {}
                rs_tiles = {}

                def emit_rs(ci):
                    a, nq = RS_CHUNKS[ci]
                    rs_out = rsdram.tile(
                        [nq * SH, C], BF16, tag="rsout", name=f"rso{ci}"
                    )
                    if _DEBUG_NO_RS:
                        nc.sync.dma_start(rs_out[:, :], yb_tiles[ci][0 : nq * SH, :])
                    else:
                        nc.gpsimd.collective_compute(
                            "ReduceScatter",
                            mybir.AluOpType.add,
                            replica_groups=GROUPS,
                            ins=[yb_tiles[ci][:, :].opt()],
                            outs=[rs_out.opt()],
                        )
                    rs_tiles[ci] = rs_out

                def emit_epilogue(ci, final=False):
                    # Deferred one chunk: by the time this is dispatched the
                    # collective is long done, so the Pool-queue waits are
                    # pre-satisfied and never delay the next RS dispatch.
                    a, nq = RS_CHUNKS[ci]
                    rt = ybp.tile([128, nq, C], BF16, tag="rsb", bufs=2)
                    nc.gpsimd.dma_start(
                        rt, rs_tiles[ci].rearrange("(o p) m -> p o m", p=128)
                    )
                    rtf = ybp.tile([128, nq, C], F32, tag="rsf", bufs=2)
                    if final:
                        # tail: DVE is idle and ~2x faster than the Q7 copy
                        nc.vector.tensor_copy(out=rtf, in_=rt)
                    else:
                        # mid-stream: Pool engine, off the DVE exp path
                        nc.gpsimd.tensor_copy(out=rtf, in_=rt)
                    nc.gpsimd.dma_start(
                        out_ext[a * SH : (a + nq) * SH, :].rearrange(
                            "(o p) m -> p o m", p=128
                        ),
                        rtf,
                        accum_op=mybir.AluOpType.add,
                    )

                from functools import partial

                def norm_transpose(h, qr, o4):
                    # normalize per-partition: on = o[:,:96] / o[:,96]
                    # (d padded to 128 for XBAR alignment; pad zeroed)
                    on_qr = onp.tile([128, 4, 128], BF16, tag="on")
                    nc.vector.memset(on_qr[:, :, 96:128], 0.0)
                    r4 = misc.tile([128, 4, 1], F32, tag="r")
                    nc.vector.reciprocal(r4, o4[:, :, 96:97])
                    r4_ap = r4[:, :, :]
                    r4_bc = bass.AP(
                        tensor=r4_ap.tensor,
                        offset=r4_ap.offset,
                        ap=list(r4_ap.ap[:-1]) + [[0, 96]],
                    )
                    nc.vector.tensor_tensor(
                        on_qr[:, :, 0:96],
                        o4[:, :, 0:96],
                        r4_bc,
                        mybir.AluOpType.mult,
                    )
                    # oT via the DMA XBAR transpose (SP queue; dispatch is
                    # async from the transfer)
                    nc.sync.dma_start_transpose(
                        onT_sb[h][:, qr * 4 : (qr + 1) * 4, :], on_qr
                    )

                def proj_chunk(qr, j):
                    # projection: y[n,:] = sum_h onT_h[:, n]^T @ wp_h
                    nchunk = qr * 4 + j
                    ci = QR_CHUNK[qr]
                    a, nq = RS_CHUNKS[ci]
                    if j == 0 and qr == a:
                        yb_tiles[ci] = ydram.tile(
                            [nq * QR, C], BF16, tag="yb", name=f"yb{ci}"
                        )
                    yp = ps.tile([128, 1024], F32, tag="ps")
                    for hh in range(2):
                        for lo, hi in [(0, 512), (512, 768)]:
                            nc.tensor.matmul(
                                yp[:, lo:hi],
                                lhsT=onT_sb[hh][0:96, nchunk, :],
                                rhs=wp_sb[:96, hh, lo:hi],
                                start=(hh == 0),
                                stop=(hh == 1),
                            )
                    y_sb = ybp.tile([128, C], BF16, tag="y")
                    nc.vector.tensor_copy(out=y_sb, in_=yp[:, :C])
                    lo = (qr - a) * QR + j * 128
                    nc.sync.dma_start(yb_tiles[ci][lo : lo + 128, :], y_sb)

                def attention(h, unit_plan, carry_in=None):
                    """Flash attention for head h over all q-ranges; emits
                    closures from unit_plan[(qr, kcp)] between score groups.
                    Each q-range's finalize (normalize/transpose/proj/RS) is
                    deferred into the NEXT q-range's score stream."""
                    unit_plan = unit_plan or {}
                    stages = dict(carry_in or {})
                    for qr in range(N_QR):
                        # all 4 q128-chunk accumulators share ONE psum bank;
                        # the first matmul's start=True zeroes the whole 2KB
                        # zero-region, later j's first matmuls ride on it
                        o4 = op.tile([128, 4, 97], F32, tag="o")

                        def emit_pv(kcp, p_t):
                            for kk in range(2):
                                kc = 2 * kcp + kk
                                for j in range(4):
                                    nc.tensor.matmul(
                                        o4[:, j, 0:97],
                                        lhsT=p_t[:, kk, j * 128 : (j + 1) * 128],
                                        rhs=vp_sb[h][:, kc, :],
                                        start=(kcp == 0 and kk == 0 and j == 0),
                                        stop=(kcp == N_KC // 2 - 1 and kk == 1),
                                        skip_group_check=True,
                                    )

                        # PV runs one kcp group behind the score/exp stream:
                        # the in-order PE queue would otherwise stall on
                        # exp(g) between QK(g) and PV(g)
                        pv_prev = None
                        for kcp in range(N_KC // 2):
                            sp = ps.tile([128, 2, 512], F32, tag="ps")
                            for kk in range(2):
                                kc = 2 * kcp + kk
                                nc.tensor.matmul(
                                    sp[:, kk, :],
                                    lhsT=kT_sb[h][:, kc * 128 : (kc + 1) * 128],
                                    rhs=qT_sb[h][:, qr * QR : (qr + 1) * QR],
                                    start=True,
                                    stop=True,
                                )
                            p_t = pp.tile([128, 2, 512], BF16, tag="p")
                            if kcp in DVE_EXP_KCP[h]:
                                nc.vector.tensor_scalar(
                                    out=p_t[:, :, :].bitcast(mybir.dt.int16),
                                    in0=sp,
                                    scalar1=SCH_SCALE,
                                    scalar2=SCH_BIAS,
                                    op0=mybir.AluOpType.mult,
                                    op1=mybir.AluOpType.add,
                                )
                            else:
                                nc.scalar.activation(
                                    p_t, sp, mybir.ActivationFunctionType.Exp
                                )
                            if pv_prev is not None:
                                emit_pv(kcp - 1, pv_prev)
                            pv_prev = p_t
                            for u in stages.pop(kcp, ()):
                                u()
                            for u in unit_plan.pop((qr, kcp), ()):
                                u()
                        emit_pv(N_KC // 2 - 1, pv_prev)

                        stages = {0: [partial(norm_transpose, h, qr, o4)]}
                        if h == 1:
                            for j in range(4):
                                stages.setdefault(2 + 2 * j, []).append(
                                    partial(proj_chunk, qr, j)
                                )
                            ci = QR_CHUNK[qr]
                            if qr == RS_CHUNKS[ci][0] + RS_CHUNKS[ci][1] - 1:
                                stages.setdefault(11, []).append(partial(emit_rs, ci))
                                # previous chunk's epilogue: its RS is done by
                                # now, so the waits don't stall the Pool queue
                                if ci > 0:
                                    stages.setdefault(13, []).append(
                                        partial(emit_epilogue, ci - 1)
                                    )
                                if ci == len(RS_CHUNKS) - 1:
                                    stages.setdefault(14, []).append(
                                        partial(emit_epilogue, ci, True)
                                    )
                    return stages

                # head-0 QKV emitted just-in-time inside head-0's own
                # attention sweep; paired v units serve both heads.
                # v_un[n] feeds PV at kcp=n (PV runs one group late, so
                # stage n-1 suffices; v_un[0] rides at stage 0, off the
                # first-exp critical path); k-half n feeds scores at kcp=2n;
                # q-half n feeds q-range n
                u0 = qk_units(0)
                q_un = [u0[4 * i + j] for i in range(4) for j in (0, 1)]
                k_un = [u0[4 * i + j] for i in range(4) for j in (2, 3)]
                v_un = v_units()
                plan = {}

                def put(qr, kcp, u):
                    plan.setdefault((qr, kcp), []).append(u)

                for n in range(16):
                    put(0, max(n - 1, 0), v_un[n])
                for n in range(1, 8):
                    put(0, 2 * n - 2, k_un[n])
                for n in range(1, 8):
                    put(n - 1, 5, q_un[n])
                u1 = qk_units(1)
                if _INTERLEAVE_QKV1:
                    slots = [
                        (qr, kcp)
                        for qr in range(1, 8)
                        for kcp in (4, 7, 10, 13, 15)
                    ]
                    assert len(slots) >= len(u1)
                    for u, s in zip(u1, slots):
                        put(s[0], s[1], u)
                    u1 = []
                for u in (q_un[0], k_un[0], *u1):
                    u()
                # head 0's last finalize stages carry into head 1's stream;
                # head 1's last stages (norm + proj tail + final RS +
                # epilogues) flush after the loop
                carry = attention(0, plan)
                last = attention(1, None, carry_in=carry)
                for kcp in sorted(last):
                    for u in last[kcp]:
                        u()

    if _HOIST:
        _hoist_waits(nc)
    return nc


_NC_CACHE = None


def _get_nc():
    global _NC_CACHE
    if _NC_CACHE is None:
        _NC_CACHE = build()
    return _NC_CACHE


def make_in_maps(x, w_qkv, w_proj, b_proj):
    x = np.asarray(x, dtype=np.float32)
    w_qkv = np.asarray(w_qkv, dtype=np.float32).reshape(C, 3, H, DH)
    w_proj = np.asarray(w_proj, dtype=np.float32)
    b_proj = np.asarray(b_proj, dtype=np.float32)
    scale = DH ** -0.5
    NCC = C // 128

    def chunked(w):  # [C, M] -> [128, NCC, M] (partition-contiguous strips)
        return np.ascontiguousarray(
            w.reshape(NCC, 128, -1).transpose(1, 0, 2)
        ).astype(BF16_NP)

    xT_b = [chunked(np.ascontiguousarray(x[b].T)) for b in range(B)]
    in_maps = []
    for i in range(N_CORES):
        b = i // 4
        h0 = HEADS_PER_CORE * (i % 4)
        sl = slice(h0, h0 + HEADS_PER_CORE)
        wq_i = chunked(w_qkv[:, 0, sl, :].reshape(C, HD) * scale)
        wk_i = chunked(w_qkv[:, 1, sl, :].reshape(C, HD))
        wv_i = chunked(w_qkv[:, 2, sl, :].reshape(C, HD))
        wp_i = np.ascontiguousarray(
            w_proj.reshape(H, DH, C)[sl].reshape(HEADS_PER_CORE, DH, C)
            .transpose(1, 0, 2)
        ).astype(BF16_NP)  # [96, 2, C]
        in_maps.append(
            {
                "xT": xT_b[b],
                "wq": wq_i,
                "wk": wk_i,
                "wv": wv_i,
                "wp": wp_i,
                "bias": b_proj,
            }
        )
    return in_maps


def assemble(results):
    # rank r of batch-group b holds, per RS chunk (a, nq) covering global
    # rows [a*512, (a+nq)*512), the piece [a*512 + r*nq*128, ... + nq*128)
    # at out_ext rows [a*128, (a+nq)*128)
    out = np.empty((B, N, C), dtype=np.float32)
    for i in range(N_CORES):
        b, r = i // 4, i % 4
        shard = results[i]["out"]
        for a, nq in RS_CHUNKS:
            rows = nq * SH
            lo = a * QR + r * rows
            out[b, lo : lo + rows, :] = shard[a * SH : a * SH + rows]
    return out


def kernel(x, w_qkv, w_proj, b_proj):
    nc = _get_nc()
    in_maps = make_in_maps(x, w_qkv, w_proj, b_proj)
    res = run_bass_kernel_spmd(nc, in_maps, core_ids=list(range(N_CORES)))
    return assemble(res.results)
